# revision 24
# baseline (speedup 1.0000x reference)
"""Linear attention (B=4, S=4096, D=1024, H=16) on 8 TRN2 NeuronCores.

Sharding: core = (batch, head-half): each core handles one batch's 8 heads.
 - x is host-transposed + pre-tiled so every DMA is contiguous per
   partition and both matmul orientations need no on-device transpose.
 - Wqkv column-sharded per head-half; Wo row-sharded; host sums the two
   partial y's per batch (row-parallel unshard). y is emitted bf16.

Per-core dataflow (S=4096 in 8 blocks of 512 tokens), all matmuls bf16
(fp32 PSUM accumulate):
  startup: ~3.5us of dummy matmuls warm the PE clock gate (HAM) during
      the DMA wait; wqkv + x-block-0 stream as 16 interleaved per-ko
      pieces on the sync DMA path (16 HW queues, rings complete in
      enqueue order) so QT starts consuming at ~9us.
  phase A: qkv projection:
      QT [512f, S] feature-major (block 0 k-outer to trickle-consume the
      arriving chunks, 4 PSUM banks; later blocks f-outer, 1 bank)
      K,V [S, 512f] token-major   (lhsT=xt, rhs=Wkv)
      elu(x)+1 = min(exp(x),1) + relu(x): ACT Exp + DVE max + DVE stt
  phase B: per head-pair [KV | K_sum^T] PSUM accumulation over all tokens
      (vst carries a ones column per pair so one matmul does both).
      Block 7 runs K/V+B+conversions BEFORE its QT so the C-phase
      prerequisites finalize under QT's 32 matmuls (no boundary bubble).
  phase C (pipelined with D of the previous block), pair-packed:
      psc[128,s] = blockdiag(KV_h0, KV_h1)^T @ QT_pair; ACT evict to outu
      norm rows via zero-padded M=32 col-tiled pair matmuls (concurrent,
      one PSUM bank); rcp = reciprocal_approx_fast (ONE custom DVE op --
      no ACT Ln/Exp, so the activation table is loaded exactly once) +
      DVE cast to fp32r; psn/psr share one 4-buf PSUM ring
      rcpb: per-pair K=2 row-tiled matmuls against ones2 (concurrent)
      broadcast the two rcp rows to the matching 64-partition halves
      outT = outu * rcpb (one full-width DVE mult per pair, bf16)
  phase D: y[s,:] (+)= outT^T @ Wo, skewed one block behind C; PSUM->SBUF
      evicts alternate ACT/DVE; y DMA'd out bf16 in quarter-blocks.
"""

import numpy as np

import concourse.bacc as bacc
import concourse.mybir as mybir
import concourse.tile as tile
from concourse.bass_utils import run_bass_kernel_spmd

F32 = mybir.dt.float32
F32R = mybir.dt.float32r
BF16 = mybir.dt.bfloat16

P = 128
B, S, D = 4, 4096, 1024
H = 16
HD = 64

FSH = 512            # features per core for each of Q, K, V (8 heads)
KSUB = D // P        # 8 contraction subtiles
SBLK = 512           # tokens per block
NBLK = S // SBLK     # 8 blocks
TSUB = SBLK // P     # 4 token subtiles per block
NPAIR = 4            # head pairs per core

_NC_CACHE = None


def build():
    import contextlib

    nc = bacc.Bacc(target_bir_lowering=False)
    # host-pretiled: xtb[p, j, ko, s] = x[j*512+s, ko*128+p]
    xtb = nc.dram_tensor("xtb", [P, NBLK, KSUB, SBLK], BF16, kind="ExternalInput")
    # wqkv[p, ko, f] (f = q512|k512|v512)
    wqkv = nc.dram_tensor("wqkv", [P, KSUB, 3 * FSH], BF16, kind="ExternalInput")
    # wo[p, fo, n]
    wo = nc.dram_tensor("wo", [P, FSH // P, D], BF16, kind="ExternalInput")
    ones2 = nc.dram_tensor("ones2", [P, P], F32R, kind="ExternalInput")
    y = nc.dram_tensor("y", [S, D], BF16, kind="ExternalOutput")
    y_r2 = y.rearrange(
        "(j th t p) n -> j p th t n", th=TSUB // 2, t=2, p=P
    )  # [8, 128, 2, 2, 1024]

    with tile.TileContext(nc) as tc:
        with contextlib.ExitStack() as ctx:
            const = ctx.enter_context(tc.tile_pool(name="const", bufs=1))
            wpool = ctx.enter_context(tc.tile_pool(name="wpool", bufs=1))
            qtpool = ctx.enter_context(tc.tile_pool(name="qtpool", bufs=1))

            # wqkv in 8 per-ko chunks, all on the sync DMA path (its dynamic
            # queue spreads packets over 16 HW queues at ~300GB/s; the scalar
            # and gpsimd dynamic queues are single-queue ~23GB/s). Per-queue
            # rings complete in enqueue order, so interleaving weight and x
            # chunks in consumption order lets QT start ~9us in and trickle.
            wq_sb = [
                wpool.tile([P, 1, 3 * FSH], BF16, name=f"wq{c}") for c in range(8)
            ]
            wo_sb = wpool.tile([P, FSH // P, D], BF16)
            ones2_fr = const.tile([P, P], F32R)
            nc.gpsimd.dma_start(out=ones2_fr, in_=ones2[:])

            qt_sb = qtpool.tile([P, FSH // P, S], BF16)   # feature-major Q
            # per-pair block-diagonal [[KV_h0, 0], [0, KV_h1]] (128x128)
            lhsT2_sb = [
                qtpool.tile([P, P], BF16, name=f"lhsT2{p}") for p in range(NPAIR)
            ]
            # per-pair [Ksum_h0 | Ksum_h1 | zeros] (128 x 32)
            ksumpad_sb = [
                qtpool.tile([P, 32], BF16, name=f"ksp{p}") for p in range(NPAIR)
            ]
            def wqk(k, sl):
                return wq_sb[k][:, 0, sl]

            # HAM warm-up: ~3.5us of dummy matmuls during the initial DMA
            # wait flips the PE clock gate to 2.4GHz before real work lands.
            with (
                tc.tile_pool(name="warm", bufs=1) as warm_pool,
                tc.tile_pool(name="warmps", bufs=2, space="PSUM") as warm_ps,
            ):
                wsrc = warm_pool.tile([P, 320], BF16)
                nc.vector.memset(wsrc, 0.0)
                for i in range(12):
                    wps = warm_ps.tile([P, 320], F32, tag="w")
                    nc.tensor.matmul(
                        wps, wsrc[:, 0:P], wsrc, start=True, stop=True
                    )

            # zero-fill early (no kvps dependency) so the phase-boundary only
            # pays for the data copies.
            for p_ in range(NPAIR):
                nc.vector.memset(ksumpad_sb[p_], 0.0)
                nc.vector.memset(lhsT2_sb[p_], 0.0)

            # ---------------- phase A + B ----------------
            with contextlib.ExitStack() as abctx:
                xpool = abctx.enter_context(tc.tile_pool(name="xin", bufs=3))
                x0pool = abctx.enter_context(tc.tile_pool(name="x0in", bufs=1))
                stpool = abctx.enter_context(tc.tile_pool(name="stage", bufs=3))
                pa_ps = abctx.enter_context(
                    tc.tile_pool(name="paps", bufs=4, space="PSUM")
                )
                etpool = abctx.enter_context(tc.tile_pool(name="etmp", bufs=4))
                # kvps tiles are allocated lazily (after block 0's
                # interleaved QT+KV section) so its temporary 4-bank pool
                # fits alongside pa_ps in the 8-bank PSUM budget.
                kvps = []

                # block 0 as 8 per-ko tiles, DMA'd interleaved with the
                # matching weight chunk so QT's k-outer loop consumes pieces
                # as they land; wo rides behind (only needed in phase D).
                xt0s = [
                    x0pool.tile([P, 1, SBLK], BF16, name=f"xt0{k}")
                    for k in range(KSUB)
                ]
                for k in range(KSUB):
                    nc.sync.dma_start(out=wq_sb[k], in_=wqkv[:, k : k + 1])
                    nc.sync.dma_start(out=xt0s[k], in_=xtb[:, 0, k : k + 1])
                nc.sync.dma_start(out=wo_sb, in_=wo[:])

                def emit_qt(j, xk):
                    # QT: block 0 runs k-outer (consumes weight chunks as
                    # they land, 4 PSUM banks accumulate the 4 feature blocks
                    # in parallel); later blocks run f-outer so only one bank
                    # is needed at a time (k-outer there stalls ~0.9us/block
                    # waiting for all 4 banks to free at once).
                    for f in range(FSH // P):
                        ps = pa_ps.tile([P, SBLK], F32, tag="pa", name="qps")
                        for k in range(KSUB):
                            nc.tensor.matmul(
                                ps,
                                wqk(k, slice(f * P, (f + 1) * P)),
                                xk(k),
                                start=(k == 0),
                                stop=(k == KSUB - 1),
                            )
                        e = etpool.tile([P, SBLK], F32, tag="e")
                        nc.scalar.activation(
                            out=e, in_=ps,
                            func=mybir.ActivationFunctionType.Exp,
                        )
                        r = etpool.tile([P, SBLK], F32, tag="r")
                        nc.vector.tensor_scalar_max(r, ps, 0.0)
                        nc.vector.scalar_tensor_tensor(
                            out=qt_sb[:, f, j * SBLK : (j + 1) * SBLK],
                            in0=e,
                            scalar=1.0,
                            in1=r,
                            op0=mybir.AluOpType.min,
                            op1=mybir.AluOpType.add,
                        )

                def emit_kvb(j, xk):
                    # K, V token-major per 128-token subtile.
                    kst = stpool.tile([P, TSUB, FSH], BF16, tag="kst")
                    vst = stpool.tile([P, TSUB, NPAIR, P + 1], BF16, tag="vst")
                    nc.vector.memset(vst[:, :, :, P : P + 1], 1.0)
                    for t in range(TSUB):
                        psk = pa_ps.tile([P, FSH], F32, tag="pa")
                        psv = pa_ps.tile([P, FSH], F32, tag="pa")
                        for k in range(KSUB):
                            xtk = xk(k)[:, t * P : (t + 1) * P]
                            nc.tensor.matmul(
                                psk,
                                xtk,
                                wqk(k, slice(FSH, 2 * FSH)),
                                start=(k == 0),
                                stop=(k == KSUB - 1),
                            )
                            nc.tensor.matmul(
                                psv,
                                xtk,
                                wqk(k, slice(2 * FSH, 3 * FSH)),
                                start=(k == 0),
                                stop=(k == KSUB - 1),
                            )
                        e = etpool.tile([P, SBLK], F32, tag="e")
                        nc.scalar.activation(
                            out=e, in_=psk,
                            func=mybir.ActivationFunctionType.Exp,
                        )
                        r = etpool.tile([P, SBLK], F32, tag="r")
                        nc.vector.tensor_scalar_max(r, psk, 0.0)
                        nc.vector.scalar_tensor_tensor(
                            out=kst[:, t, :],
                            in0=e,
                            scalar=1.0,
                            in1=r,
                            op0=mybir.AluOpType.min,
                            op1=mybir.AluOpType.add,
                        )
                        nc.scalar.copy(out=vst[:, t, :, 0:P], in_=psv)

                    # phase B: accumulate [KV | K_sum^T] into persistent
                    # psums. pair-outer so in the last block each pair's
                    # state finalizes early and its SBUF conversion (DVE)
                    # overlaps the remaining pairs' accumulation on the PE.
                    first = False
                    last = j == NBLK - 1
                    for p_ in range(NPAIR):
                        for t in range(TSUB):
                            nc.tensor.matmul(
                                kvps[p_],
                                kst[:, t, p_ * P : (p_ + 1) * P],
                                vst[:, t, p_, :],
                                start=(first and t == 0),
                                stop=(last and t == TSUB - 1),
                            )
                        if last:
                            nc.vector.tensor_copy(
                                out=ksumpad_sb[p_][0:HD, 0:1],
                                in_=kvps[p_][0:HD, P : P + 1],
                            )
                            nc.vector.tensor_copy(
                                out=ksumpad_sb[p_][HD:P, 1:2],
                                in_=kvps[p_][HD:P, P : P + 1],
                            )
                            nc.vector.tensor_copy(
                                out=lhsT2_sb[p_][0:HD, 0:HD],
                                in_=kvps[p_][0:HD, 0:HD],
                            )
                            nc.vector.tensor_copy(
                                out=lhsT2_sb[p_][HD:P, HD:P],
                                in_=kvps[p_][HD:P, HD:P],
                            )

                def emit_block0():
                    # block 0 is DMA-gated: run QT AND the first two K/V
                    # token-subtiles k-outer together, consuming each
                    # (weight, x) chunk pair fully as it lands (8 matmuls
                    # per chunk ~= the arrival cadence). QT's 4 banks live
                    # in a temporary pool that closes before kvps allocates.
                    def xk(k):
                        return xt0s[k][:, 0, :]

                    kst = stpool.tile([P, TSUB, FSH], BF16, tag="kst")
                    vst = stpool.tile([P, TSUB, NPAIR, P + 1], BF16, tag="vst")
                    nc.vector.memset(vst[:, :, :, P : P + 1], 1.0)

                    def elu_k(t, psk):
                        e = etpool.tile([P, SBLK], F32, tag="e")
                        nc.scalar.activation(
                            out=e, in_=psk,
                            func=mybir.ActivationFunctionType.Exp,
                        )
                        r = etpool.tile([P, SBLK], F32, tag="r")
                        nc.vector.tensor_scalar_max(r, psk, 0.0)
                        nc.vector.scalar_tensor_tensor(
                            out=kst[:, t, :],
                            in0=e,
                            scalar=1.0,
                            in1=r,
                            op0=mybir.AluOpType.min,
                            op1=mybir.AluOpType.add,
                        )

                    with tc.tile_pool(
                        name="pa0", bufs=4, space="PSUM"
                    ) as pa0_ps:
                        qps = [
                            pa0_ps.tile([P, SBLK], F32, tag="pa0", name=f"q0{f}")
                            for f in range(FSH // P)
                        ]
                        kvt = []
                        for t in range(2):
                            psk = pa_ps.tile([P, FSH], F32, tag="pa", name=f"pk{t}")
                            psv = pa_ps.tile([P, FSH], F32, tag="pa", name=f"pv{t}")
                            kvt.append((psk, psv))
                        for k in range(KSUB):
                            for f in range(FSH // P):
                                nc.tensor.matmul(
                                    qps[f],
                                    wqk(k, slice(f * P, (f + 1) * P)),
                                    xk(k),
                                    start=(k == 0),
                                    stop=(k == KSUB - 1),
                                )
                            for t in range(2):
                                psk, psv = kvt[t]
                                xtk = xk(k)[:, t * P : (t + 1) * P]
                                nc.tensor.matmul(
                                    psk,
                                    xtk,
                                    wqk(k, slice(FSH, 2 * FSH)),
                                    start=(k == 0),
                                    stop=(k == KSUB - 1),
                                )
                                nc.tensor.matmul(
                                    psv,
                                    xtk,
                                    wqk(k, slice(2 * FSH, 3 * FSH)),
                                    start=(k == 0),
                                    stop=(k == KSUB - 1),
                                )
                        for f in range(FSH // P):
                            e = etpool.tile([P, SBLK], F32, tag="e")
                            nc.scalar.activation(
                                out=e, in_=qps[f],
                                func=mybir.ActivationFunctionType.Exp,
                            )
                            r = etpool.tile([P, SBLK], F32, tag="r")
                            nc.vector.tensor_scalar_max(r, qps[f], 0.0)
                            nc.vector.scalar_tensor_tensor(
                                out=qt_sb[:, f, 0:SBLK],
                                in0=e,
                                scalar=1.0,
                                in1=r,
                                op0=mybir.AluOpType.min,
                                op1=mybir.AluOpType.add,
                            )
                        for t in range(2):
                            psk, psv = kvt[t]
                            elu_k(t, psk)
                            nc.scalar.copy(out=vst[:, t, :, 0:P], in_=psv)

                    # pa0 closed: now the persistent KV psum banks fit
                    kvps_pool = abctx.enter_context(
                        tc.tile_pool(name="kvps", bufs=1, space="PSUM")
                    )
                    kvps.extend(
                        kvps_pool.tile([P, P + 1], F32, tag=f"kv{p}", name=f"kv{p}")
                        for p in range(NPAIR)
                    )

                    for t in range(2, TSUB):
                        psk = pa_ps.tile([P, FSH], F32, tag="pa")
                        psv = pa_ps.tile([P, FSH], F32, tag="pa")
                        for k in range(KSUB):
                            xtk = xk(k)[:, t * P : (t + 1) * P]
                            nc.tensor.matmul(
                                psk,
                                xtk,
                                wqk(k, slice(FSH, 2 * FSH)),
                                start=(k == 0),
                                stop=(k == KSUB - 1),
                            )
                            nc.tensor.matmul(
                                psv,
                                xtk,
                                wqk(k, slice(2 * FSH, 3 * FSH)),
                                start=(k == 0),
                                stop=(k == KSUB - 1),
                            )
                        elu_k(t, psk)
                        nc.scalar.copy(out=vst[:, t, :, 0:P], in_=psv)

                    for p_ in range(NPAIR):
                        for t in range(TSUB):
                            nc.tensor.matmul(
                                kvps[p_],
                                kst[:, t, p_ * P : (p_ + 1) * P],
                                vst[:, t, p_, :],
                                start=(t == 0),
                                stop=False,
                            )

                for j in range(NBLK):
                    if j == 0:
                        emit_block0()
                        continue
                    if True:
                        xt = xpool.tile([P, KSUB, SBLK], BF16, tag="xt")
                        nc.sync.dma_start(out=xt, in_=xtb[:, j])

                        def xk(k, _xt=xt):
                            return _xt[:, k, :]

                    if j < NBLK - 1:
                        emit_qt(j, xk)
                        emit_kvb(j, xk)
                    else:
                        # last block: K/V+B+conversions first so the C-phase
                        # prerequisites finalize while QT's 32 matmuls keep
                        # the PE busy -- no phase-boundary bubble.
                        emit_kvb(j, xk)
                        emit_qt(j, xk)

            # ---------------- phase C + D ----------------
            # psn and the 4 psr broadcast banks share one 4-buf PSUM ring:
            # per block the ring carries [psn, psr0..psr3]; psn(j+1) lands on
            # psr(j,0)'s slot after its apply has read it.
            with (
                tc.tile_pool(name="pcps", bufs=2, space="PSUM") as pc_ps,
                tc.tile_pool(name="pnr", bufs=4, space="PSUM") as pnr_ps,
                tc.tile_pool(name="pyps", bufs=2, space="PSUM") as py_ps,
                tc.tile_pool(name="cd", bufs=2) as cdpool,
                tc.tile_pool(name="ou", bufs=2) as oupool,
                tc.tile_pool(name="rc", bufs=2) as rcpool,
                tc.tile_pool(name="yout", bufs=4) as ypool,
            ):
                outus = {}
                rcps = {}
                outts = {}

                def c_norm(j):
                    psn = pnr_ps.tile([P, SBLK], F32, tag="pnr", name="psn")
                    for p_ in range(NPAIR):
                        nc.tensor.matmul(
                            psn[32 * p_ : 32 * p_ + 32, :],
                            ksumpad_sb[p_],
                            qt_sb[:, p_, j * SBLK : (j + 1) * SBLK],
                            start=True,
                            stop=True,
                            tile_position=(0, 32 * p_),
                        )
                    # one custom-DVE op: ~18-bit 1/x (normalizer is always
                    # >= O(1) positive, so no eps needed; the zero-padded
                    # rows produce NaN that no later op reads), then a DVE
                    # cast to fp32r (the rcpb matmul operand must be
                    # fp32r-rounded at the producer)
                    rcf = rcpool.tile([P, SBLK], F32, tag="rcf", name="rcf")
                    nc.vector.reciprocal_approx_fast(out=rcf, in_=psn)
                    rcpt = rcpool.tile([P, SBLK], F32R, tag="rc", name="rcpt")
                    with nc.allow_low_precision(
                        reason="fp32r is 32-bit; fp32r matmul operand"
                    ):
                        nc.vector.tensor_copy(out=rcpt, in_=rcf)
                    rcps[j] = rcpt

                def c_psc(j, ps_):
                    if j not in outus:
                        outus[j] = oupool.tile(
                            [P, NPAIR, SBLK], F32, tag="outu", name="outu"
                        )
                    outu = outus[j]
                    for p_ in ps_:
                        psc = pc_ps.tile([P, SBLK], F32, tag="pc", name="psc")
                        nc.tensor.matmul(
                            psc,
                            lhsT2_sb[p_],
                            qt_sb[:, p_, j * SBLK : (j + 1) * SBLK],
                            start=True,
                            stop=True,
                        )
                        nc.scalar.copy(out=outu[:, p_, :], in_=psc)

                def c_rcpb_apply(j):
                    outu = outus.pop(j)
                    rcpt = rcps.pop(j)
                    outt = cdpool.tile(
                        [P, FSH // P, SBLK], BF16, tag="outt", name="outt"
                    )
                    outts[j] = outt
                    for p_ in range(NPAIR):
                        rb = 32 * p_
                        psr = pnr_ps.tile([P, SBLK], F32, tag="pnr", name="psr")
                        nc.tensor.matmul(
                            psr,
                            ones2_fr[rb : rb + 2, :],
                            rcpt[rb : rb + 2, :],
                            start=True,
                            stop=True,
                            tile_position=(rb, 0),
                        )
                        nc.vector.tensor_tensor(
                            out=outt[:, p_, :],
                            in0=outu[:, p_, :],
                            in1=psr,
                            op=mybir.AluOpType.mult,
                        )

                def d_half(j, th):
                    outt = outts[j]
                    for t2 in range(2):
                        t = th * 2 + t2
                        ysb = ypool.tile([P, D], BF16, tag="ysb", name="ysb")
                        for nb in range(D // 512):
                            psy = py_ps.tile([P, 512], F32, tag="py", name="psy")
                            for fs in range(FSH // P):
                                nc.tensor.matmul(
                                    psy,
                                    outt[:, fs, t * P : (t + 1) * P],
                                    wo_sb[:, fs, nb * 512 : (nb + 1) * 512],
                                    start=(fs == 0),
                                    stop=(fs == FSH // P - 1),
                                )
                            # alternate evict engine: keeps both ACT and DVE
                            # under the per-block PE time
                            dst = ysb[:, nb * 512 : (nb + 1) * 512]
                            if (t2 * 2 + nb) % 2 == 0:
                                nc.scalar.copy(out=dst, in_=psy)
                            else:
                                nc.vector.tensor_copy(out=dst, in_=psy)
                        # quarter-block DMAs shorten the end-of-kernel drain
                        nc.sync.dma_start(out=y_r2[j, :, th, t2], in_=ysb)
                    if th == TSUB // 2 - 1:
                        outts.pop(j)

                for j in range(NBLK):
                    c_norm(j)
                    c_psc(j, [0, 1])
                    if j >= 1:
                        d_half(j - 1, 0)
                    c_psc(j, [2, 3])
                    c_rcpb_apply(j)
                    if j >= 1:
                        d_half(j - 1, 1)
                d_half(NBLK - 1, 0)
                d_half(NBLK - 1, 1)

    nc.compile()
    return nc


def _prep_inputs(x, Wqkv, Wo):
    import ml_dtypes

    x = np.ascontiguousarray(x, dtype=np.float32)
    Wqkv = np.ascontiguousarray(Wqkv, dtype=np.float32)
    Wo = np.ascontiguousarray(Wo, dtype=np.float32)
    ones2 = np.zeros((128, 128), dtype=np.float32)
    for k in range(4):
        ones2[32 * k, 0:64] = 1.0
        ones2[32 * k + 1, 64:128] = 1.0
    in_maps = []
    for b in range(B):
        xT = x[b].T  # [D, S]
        # xtb[p, j, ko, s] = xT[ko*128+p, j*512+s]
        xtb = np.ascontiguousarray(
            xT.reshape(KSUB, P, NBLK, SBLK).transpose(1, 2, 0, 3)
        ).astype(ml_dtypes.bfloat16)
        for hh in range(2):
            cols = slice(hh * FSH, (hh + 1) * FSH)
            wq = Wqkv[:, 0 * D :][:, cols]
            wk = Wqkv[:, 1 * D :][:, cols]
            wv = Wqkv[:, 2 * D :][:, cols]
            wqkv_sh = np.concatenate([wq, wk, wv], axis=1)  # [D, 1536]
            # [p, ko, f]
            wqkv_sh = np.ascontiguousarray(
                wqkv_sh.reshape(KSUB, P, 3 * FSH).transpose(1, 0, 2)
            ).astype(ml_dtypes.bfloat16)
            wo_sh = Wo[hh * FSH : (hh + 1) * FSH, :]  # [FSH, D]
            # [p, fo, n]
            wo_sh = np.ascontiguousarray(
                wo_sh.reshape(FSH // P, P, D).transpose(1, 0, 2)
            ).astype(ml_dtypes.bfloat16)
            in_maps.append(
                {"xtb": xtb, "wqkv": wqkv_sh, "wo": wo_sh, "ones2": ones2}
            )
    return in_maps


def kernel(x, Wqkv, Wo):
    global _NC_CACHE
    if _NC_CACHE is None:
        _NC_CACHE = build()
    nc = _NC_CACHE
    in_maps = _prep_inputs(x, Wqkv, Wo)
    res = run_bass_kernel_spmd(nc, in_maps, list(range(2 * B))).results
    y = np.empty((B, S, D), dtype=np.float32)
    for b in range(B):
        y[b] = res[2 * b]["y"].astype(np.float32) + res[2 * b + 1]["y"].astype(
            np.float32
        )
    return y
